# revision 41
# baseline (speedup 1.0000x reference)
"""Trainium2 Bass kernel for rotated-filter-bank conv + channel sort + std.

Pipeline (per image): conv(x, 12 rotated 7x7 kernels, pad 3) -> leaky_relu
-> sort over the 12 channels per pixel -> concat unbiased std as channel 12.

Strategy: pure data parallel over 8 NeuronCores (2 images each), with a
software-pipelined emission over half-images (2 super-blocks = 256 rows):
  - conv rhs [98, 32*512] bf16 built straight from DRAM by 7 by-dx DMAs
    (7 horizontal taps x 14 rows replication), prefetched one half ahead,
  - 32 bf16 matmuls (K=98, M=96=12ch*8rows, N=512) into fp32 PSUM,
  - ACT PRelu evicts PSUM -> rc fp16 [96, 8192] per super-block,
  - 12 per-channel DMAs per super-block gather rc into channel-planar
    fp16 tiles [128, 1024] (partition p = r*16 + b; DRAM output rows
    stay p-permuted and the host un-permutes),
  - channel sums: S pair-adds on DVE (fp16 2x), combining chains and
    Y2 adds on Pool in fp32, squares on ACT in fp32,
  - std = sqrt(max(Y2 - (S/sqrt12)^2, 0)/11) via ACT Square + Pool
    subtract + DVE clamp + ACT Sqrt,
  - 39-comparator sorting network on DVE, each comparator two fp16
    2x tensor_tensor ops (min, max), ping-ponging each channel between
    its slot in two of three rotating planar buffers (all finals land
    in the phase-1 buffer, wire parity prematched),
  - 2 merged 12-channel output DMAs + 1 std DMA per half image.
Outputs are fp16, upcast to fp32 on the host.
"""

import numpy as np
import ml_dtypes

KSIZE = 7
SIGMA = 3
CHANNELS = 12
H = W = 512
B = 16
N_CORES = 8
IMGS_PER_CORE = B // N_CORES  # 2
R = 8                 # output rows per block
QROWS = R + 6         # input rows per block
KDIM = QROWS * 7      # 98
MDIM = CHANNELS * R   # 96
TBLOCKS = 16          # blocks per super-block
SB_ROWS = R * TBLOCKS  # 128
NSB = H // SB_ROWS    # 4 super-blocks per image
PADW = W + 6          # 518
FSB = TBLOCKS * W     # 8192 free elems per sb in rc layout
CHW = NSB * W         # 2048 free elems per channel-planar image tile

# 39-comparator sorting network for 12 inputs (verified by 0-1 principle).
SORT_NET = [(0, 8), (1, 7), (2, 6), (3, 11), (4, 10), (5, 9),
            (0, 1), (2, 5), (3, 4), (6, 9), (7, 8), (10, 11),
            (0, 2), (1, 6), (5, 10), (9, 11),
            (0, 3), (1, 2), (4, 6), (5, 7), (8, 11), (9, 10),
            (1, 4), (3, 5), (6, 8), (7, 10),
            (1, 3), (2, 5), (6, 9), (8, 10),
            (2, 3), (4, 5), (6, 7), (8, 9),
            (4, 6), (5, 7),
            (3, 4), (5, 6), (7, 8)]


def _rotated_bank(kernel2d):
    """Replicates the reference affine_grid + grid_sample rotation in numpy."""
    lin = np.linspace(-1.0, 1.0, KSIZE)
    xs, ys = np.meshgrid(lin, lin)
    thetas = np.arange(CHANNELS) * np.pi / CHANNELS
    c = np.cos(thetas)[:, None, None]
    s = np.sin(thetas)[:, None, None]
    gx = (c * xs - s * ys).astype(np.float32)
    gy = (s * xs + c * ys).astype(np.float32)

    ix = (gx + np.float32(1.0)) * np.float32(0.5) * np.float32(KSIZE - 1)
    iy = (gy + np.float32(1.0)) * np.float32(0.5) * np.float32(KSIZE - 1)
    ix0 = np.floor(ix)
    iy0 = np.floor(iy)
    ix1 = ix0 + np.float32(1.0)
    iy1 = iy0 + np.float32(1.0)
    wx1 = ix - ix0
    wx0 = np.float32(1.0) - wx1
    wy1 = iy - iy0
    wy0 = np.float32(1.0) - wy1

    def gather(iyc, ixc):
        valid = ((ixc >= 0) & (ixc <= KSIZE - 1) & (iyc >= 0)
                 & (iyc <= KSIZE - 1)).astype(np.float32)
        iyi = np.clip(iyc, 0, KSIZE - 1).astype(np.int32)
        ixi = np.clip(ixc, 0, KSIZE - 1).astype(np.int32)
        return kernel2d[iyi, ixi] * valid

    rot = (gather(iy0, ix0) * wy0 * wx0 + gather(iy0, ix1) * wy0 * wx1 +
           gather(iy1, ix0) * wy1 * wx0 + gather(iy1, ix1) * wy1 * wx1)
    return rot.astype(np.float32)  # (12, 7, 7)


def _bf16(a):
    return np.asarray(a, np.float32).astype(ml_dtypes.bfloat16)


_RUNNER_CACHE = {}


def _build_runner(iters=1):
    import bass_rust
    import concourse.tile as tile
    from concourse import bacc, mybir

    F32 = mybir.dt.float32
    BF16 = mybir.dt.bfloat16
    FP16 = mybir.dt.float16
    Act = mybir.ActivationFunctionType
    Alu = mybir.AluOpType

    def V(pairs):
        return bass_rust.VecI64Pair(pairs)

    nc = bacc.Bacc("TRN2", target_bir_lowering=False, debug=False,
                   enable_asserts=False, num_devices=N_CORES)

    xb_d = nc.dram_tensor("xb", [IMGS_PER_CORE, PADW, PADW], BF16,
                          kind="ExternalInput")
    w0_d = nc.dram_tensor("w0", [KDIM, MDIM], BF16, kind="ExternalInput")
    # y rows within each 128-row super-block are stored in permuted order
    # p = r*16 + b  (image row = b*8 + r); the host un-permutes.
    y_d = nc.dram_tensor("y", [IMGS_PER_CORE, CHANNELS + 1, H, W], FP16,
                         kind="ExternalOutput")

    HCW = 2 * W    # 1024: columns per channel in a half-image planar tile
    HSB = 2        # super-blocks per half image
    HFREE = HSB * FSB  # 16384: rhs free elems per half image

    # wire write-count parity in SORT_NET: wires 3..8 are odd (init in
    # the phase-0 buffer), wires 0-2 and 9-11 even (init in phase-1).
    # All finals land in the phase-1 buffer at slot == wire.
    INIT_PHASE = [1, 1, 1, 0, 0, 0, 0, 0, 0, 1, 1, 1]

    with tile.TileContext(nc) as tc:
        with tc.tile_pool(name="const", bufs=1) as cpool, \
             tc.tile_pool(name="rhs", bufs=2) as rpool, \
             tc.tile_pool(name="rc", bufs=1) as rcpool, \
             tc.tile_pool(name="chi", bufs=1) as chipool, \
             tc.tile_pool(name="sm", bufs=2) as smpool, \
             tc.tile_pool(name="st", bufs=2) as stpool, \
             tc.tile_pool(name="pc", bufs=2, space="PSUM") as pcpool:

            w0 = cpool.tile([KDIM, MDIM], BF16, tag="w0")
            nc.sync.dma_start(w0[:], w0_d.ap())

            # three 12-slot channel-planar buffers; each half uses a
            # (phase0, phase1) pair, rotating so the out-DMA of half h
            # (reading its phase-1 buffer) never collides with half h+1's
            # gathers: h:(A,B) h+1:(C,A) h+2:(B,C) ...
            chbufs = [chipool.tile([SB_ROWS, CHANNELS * HCW], FP16,
                                   name=f"ch{n}", tag=f"ch{n}")
                      for n in "ABC"]
            ROT = [(0, 1), (2, 0), (1, 2)]

            def mk(tag, n=1, dt=FP16):
                t = smpool.tile([SB_ROWS, HCW], dt, name=tag, tag=tag,
                                bufs=n)
                return t[:, :]

            def emit_rhs(img, half):
                rhs = rpool.tile([KDIM, HFREE], BF16, tag="rhs")
                for dx in range(7):
                    src = xb_d.ap().copy()
                    src.offset = (img * PADW
                                  + half * HSB * SB_ROWS) * PADW + dx
                    src.ap = V([[PADW, QROWS], [R * PADW, 2 * TBLOCKS],
                                [1, W]])
                    dst = rhs[:, :].copy()
                    dst.offset = dx * HFREE
                    dst.ap = V([[7 * HFREE, QROWS], [W, 2 * TBLOCKS],
                                [1, W]])
                    nc.sync.dma_start(dst, src)
                return rhs

            def emit_conv(img, half, rhs, p0, p1):
                for hs in range(HSB):
                    rc = rcpool.tile([MDIM, FSB], FP16, tag="rc")
                    for th in range(TBLOCKS // 4):
                        pc = pcpool.tile([MDIM, 4 * W], F32, tag="pc")
                        for hh in range(4):
                            t = (hs * TBLOCKS + 4 * th + hh) * W
                            nc.tensor.matmul(
                                pc[:, hh * W:(hh + 1) * W],
                                w0[:], rhs[:, t:t + W],
                                start=True, stop=True)
                        nc.scalar.activation(
                            rc[:, th * 4 * W:(th + 1) * 4 * W],
                            pc[:], Act.Prelu, alpha=0.01)

                    # per-channel gathers, p = r*16 + b (partition dim
                    # must lead the SBUF-side APs):
                    # buf[r*16+b, c*HCW + hs*512 + w] = rc[c*8+r, b*512+w]
                    for c in range(CHANNELS):
                        buf = (p0, p1)[INIT_PHASE[c]]
                        gsrc = rc[:, :].copy()
                        gsrc.offset = c * R * FSB
                        gsrc.ap = V([[FSB, R], [W, TBLOCKS], [1, W]])
                        gdst = buf[:, :].copy()
                        gdst.offset = c * HCW + hs * W
                        gdst.ap = V([[CHANNELS * HCW, SB_ROWS], [1, W]])
                        eng = nc.sync if c % 2 == 0 else nc.scalar
                        eng.dma_start(gdst, gsrc)

            def emit_sums(img, half, p0, p1):
                bufs = (p0, p1)
                chs0 = [bufs[INIT_PHASE[c]][:, c * HCW:(c + 1) * HCW]
                        for c in range(CHANNELS)]

                # S: pair adds on DVE (2x), combining chains on Pool
                ps = []
                shalves = []
                for i in range(0, CHANNELS, 2):
                    p = mk(f"p{(i // 2) % 2}")
                    nc.vector.tensor_tensor(p, chs0[i], chs0[i + 1], Alu.add)
                    ps.append(p)
                    if len(ps) == 3:
                        h1 = mk("ci", 2, F32)
                        nc.vector.tensor_tensor(h1, ps[0], ps[1], Alu.add)
                        h2 = mk("co", 2, F32)
                        nc.vector.tensor_tensor(h2, h1, ps[2], Alu.add)
                        shalves.append(h2)
                        ps = []
                S = mk("sfin", 1, F32)
                nc.vector.tensor_tensor(S, shalves[0], shalves[1], Alu.add)

                # Y2: squares on ACT, adds on Pool
                qs = []
                qhalves = []
                s_prev = None
                for i in range(CHANNELS):
                    s1 = mk(f"sq{i % 2}", 1, F32)
                    nc.scalar.activation(s1, chs0[i], Act.Square)
                    if i % 2 == 0:
                        s_prev = s1
                        continue
                    p = mk(f"qp{(i // 2) % 2}", 1, F32)
                    nc.vector.tensor_tensor(p, s_prev, s1, Alu.add)
                    qs.append(p)
                    if len(qs) == 3:
                        h1 = mk("ci", 2, F32)
                        nc.vector.tensor_tensor(h1, qs[0], qs[1], Alu.add)
                        h2 = mk("co", 2, F32)
                        nc.vector.tensor_tensor(h2, h1, qs[2], Alu.add)
                        qhalves.append(h2)
                        qs = []
                Y2 = mk("qfin", 1, F32)
                nc.vector.tensor_tensor(Y2, qhalves[0], qhalves[1], Alu.add)

                # t2 = (S/sqrt12)^2 on ACT; vv = Y2 - t2 on Pool
                t2 = stpool.tile([SB_ROWS, HCW], F32, tag="t2", bufs=1)
                nc.scalar.activation(t2[:], S, Act.Square,
                                     scale=float(1.0 / np.sqrt(12.0)))
                vv = stpool.tile([SB_ROWS, HCW], FP16, tag="vv", bufs=1)
                nc.vector.tensor_tensor(vv[:], Y2, t2[:], Alu.subtract)
                return vv

            def emit_sort(img, half, p0, p1):
                phase = list(INIT_PHASE)
                bufs = (p0, p1)

                def slot(c):
                    return bufs[phase[c]][:, c * HCW:(c + 1) * HCW]

                for (i, j) in SORT_NET:
                    a, b = slot(i), slot(j)
                    phase[i] ^= 1
                    phase[j] ^= 1
                    nc.vector.tensor_tensor(slot(i), a, b, Alu.min)
                    nc.vector.tensor_tensor(slot(j), a, b, Alu.max)

            def emit_clamp(st):
                # vc = max(vv, 0) at the tail of the half's DVE stream
                vc = stpool.tile([SB_ROWS, HCW], FP16, tag="vc", bufs=1)
                nc.vector.tensor_scalar_max(vc[:], st["vv"][:], 0.0)
                st["vc"] = vc

            def emit_stdfin(st):
                # sqrt + std out (after next half's evicts in the ACT stream)
                img, half = st["img"], st["half"]
                stdt = stpool.tile([SB_ROWS, HCW], FP16, tag="std", bufs=1)
                nc.scalar.activation(stdt[:], st["vc"][:], Act.Sqrt,
                                     scale=float(1.0 / 11.0))
                od = y_d.ap().copy()
                od.offset = ((img * (CHANNELS + 1) + CHANNELS) * H
                             + half * HSB * SB_ROWS) * W
                od.ap = V([[W, SB_ROWS], [SB_ROWS * W, HSB], [1, W]])
                ssrc = stdt[:, :].copy()
                ssrc.ap = V([[HCW, SB_ROWS], [W, HSB], [1, W]])
                nc.scalar.dma_start(od, ssrc)

            def emit_chout(st):
                # 2 merged DMAs per half (one per sb): all 12 sorted
                # channels from the phase-1 buffer, iterated (p, c, w);
                # DRAM rows stay in permuted p-order (host un-permutes)
                img, half = st["img"], st["half"]
                for hs in range(HSB):
                    srct = st["p1"][:, :].copy()
                    srct.offset = hs * W
                    srct.ap = V([[CHANNELS * HCW, SB_ROWS], [HCW, CHANNELS],
                                 [1, W]])
                    od = y_d.ap().copy()
                    od.offset = (img * (CHANNELS + 1) * H
                                 + (half * HSB + hs) * SB_ROWS) * W
                    od.ap = V([[W, SB_ROWS], [H * W, CHANNELS], [1, W]])
                    nc.scalar.dma_start(od, srct)

            pending = None
            halves = [(img, half) for img in range(IMGS_PER_CORE)
                      for half in range(HSB)] * iters
            rhs_next = emit_rhs(*halves[0])
            for k, (img, half) in enumerate(halves):
                p0 = chbufs[ROT[k % 3][0]]
                p1 = chbufs[ROT[k % 3][1]]
                rhs = rhs_next
                if k + 1 < len(halves):
                    rhs_next = emit_rhs(*halves[k + 1])
                if pending is not None:
                    emit_clamp(pending)
                emit_conv(img, half, rhs, p0, p1)
                if pending is not None:
                    emit_chout(pending)
                    emit_stdfin(pending)
                vv = emit_sums(img, half, p0, p1)
                emit_sort(img, half, p0, p1)
                pending = dict(img=img, half=half, vv=vv, p0=p0, p1=p1)
            emit_chout(pending)
            emit_clamp(pending)
            emit_stdfin(pending)

    nc.compile()
    return nc


def _get_runner():
    if "r" in _RUNNER_CACHE:
        return _RUNNER_CACHE["r"]

    import jax
    import numpy as _np
    from concourse import mybir
    from concourse import bass2jax
    from jax.sharding import Mesh, PartitionSpec
    from jax.experimental.shard_map import shard_map

    nc = _build_runner()
    bass2jax.install_neuronx_cc_hook()

    part_name = nc.partition_id_tensor.name if nc.partition_id_tensor else None
    in_names, out_names, out_avals, zero_shapes = [], [], [], []
    for alloc in nc.m.functions[0].allocations:
        if not isinstance(alloc, mybir.MemoryLocationSet):
            continue
        if not alloc.memorylocations:
            continue
        name = alloc.memorylocations[0].name
        if alloc.kind == "ExternalInput":
            if name != part_name:
                in_names.append(name)
        elif alloc.kind == "ExternalOutput":
            out_names.append(name)
            shape = tuple(alloc.tensor_shape)
            dtype = mybir.dt.np(alloc.dtype)
            out_avals.append(jax.core.ShapedArray(shape, dtype))
            zero_shapes.append((shape, dtype))
    n_params = len(in_names)
    all_names = in_names + out_names
    if part_name is not None:
        all_names = all_names + [part_name]
    donate = tuple(range(n_params, n_params + len(out_names)))

    def _body(*args):
        operands = list(args)
        if part_name is not None:
            operands.append(bass2jax.partition_id_tensor())
        outs = bass2jax._bass_exec_p.bind(
            *operands,
            out_avals=tuple(out_avals),
            in_names=tuple(all_names),
            out_names=tuple(out_names),
            lowering_input_output_aliases=(),
            sim_require_finite=True,
            sim_require_nnan=True,
            nc=nc,
        )
        return tuple(outs)

    devices = jax.devices()[:N_CORES]
    mesh = Mesh(_np.asarray(devices), ("core",))
    in_specs = (PartitionSpec("core"),) * (n_params + len(out_names))
    out_specs = (PartitionSpec("core"),) * len(out_names)
    sharded = jax.jit(
        shard_map(_body, mesh=mesh, in_specs=in_specs, out_specs=out_specs,
                  check_rep=False),
        donate_argnums=donate, keep_unused=True)

    def run(in_maps):
        concat_in = [
            _np.concatenate([_np.asarray(in_maps[c][nm]) for c in range(N_CORES)], axis=0)
            for nm in in_names
        ]
        concat_zeros = [_np.zeros((N_CORES * s[0], *s[1:]), d) for (s, d) in zero_shapes]
        out_arrs = sharded(*concat_in, *concat_zeros)
        out = {}
        for i, nm in enumerate(out_names):
            a = _np.asarray(out_arrs[i])
            out[nm] = a.reshape(N_CORES, *out_avals[i].shape)
        return out

    _RUNNER_CACHE["ctx"] = dict(nc=nc, in_names=in_names, out_names=out_names,
                                out_avals=out_avals, zero_shapes=zero_shapes,
                                part_name=part_name, all_names=all_names,
                                mesh=mesh, sharded=sharded)
    _RUNNER_CACHE["r"] = run
    return run


def measure_device_time(in_maps, n1=8, n2=40, reps=3):
    """Per-NEFF-execution device time via chained-donation delta timing.

    Runs the single-exec jitted function N times back-to-back, donating the
    previous output buffers as the next call's output donors, and reports
    (T(n2) - T(n1)) / (n2 - n1).
    """
    import time as _time
    import jax
    import numpy as _np

    _get_runner()
    ctx = _RUNNER_CACHE["ctx"]
    in_names = ctx["in_names"]
    zero_shapes = ctx["zero_shapes"]
    sharded = ctx["sharded"]

    concat_in = [
        _np.concatenate([_np.asarray(in_maps[c][nm]) for c in range(N_CORES)], axis=0)
        for nm in in_names
    ]
    concat_zeros = [_np.zeros((N_CORES * s[0], *s[1:]), d) for (s, d) in zero_shapes]
    dev_in = [jax.device_put(a) for a in concat_in]

    def run_n(n):
        # independent output-donor sets so the dispatches have no
        # cross-call dependencies and can pipeline on the device
        sets = [tuple(jax.device_put(z) for z in concat_zeros)
                for _ in range(n)]
        jax.block_until_ready(sets)
        t0 = _time.perf_counter()
        outs = [sharded(*dev_in, *s) for s in sets]
        jax.block_until_ready(outs)
        return _time.perf_counter() - t0

    run_n(2)  # warm
    results = {}
    for n in (n1, n2):
        best = float("inf")
        for _ in range(reps):
            best = min(best, run_n(n))
        results[n] = best
        print(f"  n={n}: {best*1e3:.2f} ms", flush=True)
    return (results[n2] - results[n1]) / (n2 - n1)


def _prep_inputs(x, kernel):
    """Host-side prep: rotate bank, build weights, pad + bf16 x."""
    rot = _rotated_bank(np.asarray(kernel, np.float32)[0, 0])

    # lhsT [98, 96]: W[q*7+dx, c*8+r] = rot[c, q-r, dx] for 0 <= q-r <= 6
    Wm = np.zeros((KDIM, MDIM), np.float32)
    for c in range(CHANNELS):
        for r in range(R):
            for dy in range(7):
                q = r + dy
                for dx in range(7):
                    Wm[q * 7 + dx, c * R + r] = rot[c, dy, dx]
    w0 = _bf16(Wm)

    # selection matrix for channel sums: sel[c*8+r, r] = 1
    sel = np.zeros((MDIM, R), np.float32)
    for c in range(CHANNELS):
        for r in range(R):
            sel[c * R + r, r] = 1.0
    selb = _bf16(sel)
    selh = sel.astype(np.float16)

    x = np.asarray(x, np.float32)
    xp = np.zeros((B, PADW, PADW), np.float32)
    xp[:, 3:3 + H, 3:3 + W] = x[:, 0]
    xb = _bf16(xp)

    in_maps = []
    for core in range(N_CORES):
        i0 = core * IMGS_PER_CORE
        in_maps.append({
            "xb": xb[i0:i0 + IMGS_PER_CORE],
            "w0": w0,
            "selb": selb,
            "selh": selh,
        })
    return in_maps


def kernel(x, kernel):
    run = _get_runner()
    in_maps = _prep_inputs(x, kernel)
    out = run(in_maps)
    y = out["y"]  # (8, 2, 13, 512, 512) fp16, rows p-permuted per sb
    y = y.reshape(B, CHANNELS + 1, NSB, R, TBLOCKS, W)
    # DRAM row index within a super-block is p = r*16 + b; image row is
    # b*8 + r -> swap the (r, b) axes while upcasting to fp32.
    y = y.transpose(0, 1, 2, 4, 3, 5).astype(np.float32)
    return np.ascontiguousarray(y.reshape(B, CHANNELS + 1, H, W))


def calibrate_iters(in_maps, iters_list=(1, 4), ncalls=8, reps=3):
    """Device-time per program body via in-NEFF repetition deltas,
    dispatched through the low-overhead shard_map rig with
    device-resident inputs and donated outputs."""
    import time as _time
    import jax
    import numpy as _np
    from concourse import mybir
    from concourse import bass2jax
    from jax.sharding import Mesh, PartitionSpec
    from jax.experimental.shard_map import shard_map

    bass2jax.install_neuronx_cc_hook()
    results = {}
    for it in iters_list:
        nc = _build_runner(iters=it)
        part_name = (nc.partition_id_tensor.name
                     if nc.partition_id_tensor else None)
        in_names, out_names, out_avals, zero_shapes = [], [], [], []
        for alloc in nc.m.functions[0].allocations:
            if not isinstance(alloc, mybir.MemoryLocationSet):
                continue
            if not alloc.memorylocations:
                continue
            name = alloc.memorylocations[0].name
            if alloc.kind == "ExternalInput":
                if name != part_name:
                    in_names.append(name)
            elif alloc.kind == "ExternalOutput":
                out_names.append(name)
                shape = tuple(alloc.tensor_shape)
                dtype = mybir.dt.np(alloc.dtype)
                out_avals.append(jax.core.ShapedArray(shape, dtype))
                zero_shapes.append((shape, dtype))
        n_params = len(in_names)
        all_names = in_names + out_names
        if part_name is not None:
            all_names = all_names + [part_name]
        donate = tuple(range(n_params, n_params + len(out_names)))

        def _body(*args, _nc=nc, _avals=tuple(out_avals),
                  _all=tuple(all_names), _out=tuple(out_names),
                  _pn=part_name):
            operands = list(args)
            if _pn is not None:
                operands.append(bass2jax.partition_id_tensor())
            outs = bass2jax._bass_exec_p.bind(
                *operands, out_avals=_avals, in_names=_all, out_names=_out,
                lowering_input_output_aliases=(),
                sim_require_finite=True, sim_require_nnan=True, nc=_nc)
            return tuple(outs)

        devices = jax.devices()[:N_CORES]
        mesh = Mesh(_np.asarray(devices), ("core",))
        nspec = (PartitionSpec("core"),) * (n_params + len(out_names))
        sharded = jax.jit(
            shard_map(_body, mesh=mesh, in_specs=nspec,
                      out_specs=(PartitionSpec("core"),) * len(out_names),
                      check_rep=False),
            donate_argnums=donate, keep_unused=True)

        concat_in = [
            _np.concatenate([_np.asarray(in_maps[c][nm])
                             for c in range(N_CORES)], axis=0)
            for nm in in_names
        ]
        concat_zeros = [_np.zeros((N_CORES * s[0], *s[1:]), d)
                        for (s, d) in zero_shapes]
        dev_in = [jax.device_put(a) for a in concat_in]

        def run_n(n):
            bufs = tuple(jax.device_put(z) for z in concat_zeros)
            jax.block_until_ready(bufs)
            t0 = _time.perf_counter()
            for _ in range(n):
                bufs = sharded(*dev_in, *bufs)
            jax.block_until_ready(bufs)
            return _time.perf_counter() - t0

        run_n(2)  # compile + warm
        best = float("inf")
        for _ in range(reps):
            best = min(best, run_n(ncalls))
        results[it] = best
        print(f"  iters={it}: best {best*1e3:.2f} ms for {ncalls} calls",
              flush=True)
    ts = sorted(results)
    dt = (results[ts[-1]] - results[ts[0]]) / (ts[-1] - ts[0]) / ncalls
    return dt
